# revision 1
# baseline (speedup 1.0000x reference)
"""GCN node classification on 8 Trainium2 NeuronCores (Bass/Tile).

Strategy (dst-sharded graph parallel), v3:
  - Nodes padded to 100352 = 8 * 12544; core c owns dst nodes
    [c*12544, (c+1)*12544)  (98 tiles of 128).
  - Per layer: each core computes xw = g_own @ W on PE; an AllGather makes
    the full [100352, F] feature table resident on every core's HBM.
  - Edges are bucketed by (dst-tile, 25088-row src window = "run"), sorted
    by src; run lengths are the max over the 8 cores (SPMD-uniform
    schedule), rounded to 16; shorter cores pad with idx 0 and zero rows
    in M. Runs are packed into one index stream per window; dma_gather
    calls of up to 2048 idxs pull source rows (int16 idx, relative to the
    window). Host-precomputed selection matrices M[e,d] = coef[e] *
    (d == dst_local[e]) are DMA-streamed from HBM (one [128,128] tile per
    chunk x run segment), and PE accumulates psum += M^T @ Y — the
    weighted segment sum. Chunks that straddle run boundaries issue one
    matmul per overlapped run. Runs flush psum into an SBUF aggregate.
    GPSIMD descriptor generation (~7 ns/idx + ~2-3 us/call) is the
    bottleneck, hence big calls and minimal index padding.
  - Self-loop term: per tile, ACT rescales the core's own xw rows
    (scale = 2*dinv^2 per node) read back from the collective input.
  - Epilogue per tile: + self + bias, + residual, erf-GELU, PE transpose,
    next layer's matmul, DMA into the next collective's input buffer.
"""
import sys

sys.path.insert(0, "/opt/trn_rl_repo")

import numpy as np

import concourse.bass as bass  # noqa: E402
import concourse.tile as tile  # noqa: E402
from concourse import bacc, mybir  # noqa: E402
from concourse.bass_utils import run_bass_kernel_spmd  # noqa: E402

NCORES = 8
F = 128          # feature width (all layers padded to 128)
TILES = 98       # dst tiles per core
OWN = TILES * 128            # 12544 nodes per core
NT = NCORES * OWN            # 100352 padded nodes
NWIN = 4
WIN = 25088                  # src window (int16-addressable, < 32768)
GCALL = 2048                 # idxs per dma_gather call
C_OUT = 40
YBUFS = 5
MBUFS = 3
NQUSE = 4


# --------------------------------------------------------------------------
# host-side schedule
# --------------------------------------------------------------------------

class Sched:
    """Shared (core-independent) schedule.

    runs:  list of dicts {q, t, R, s_lo (stream pos within q), first/last}
    calls: list of dicts {q, lo, n, chunks: [ {slot, segs: [
               {run_idx, e_lo, e_hi, m_idx, start, stop} ] } ]}
    MX:    total number of M tiles
    IDXC:  int16 idx columns (sum over calls of n/16)
    """


def make_schedule(R_tq):
    sched = Sched()
    sched.runs = []
    qlen = [0] * NWIN
    for q in range(NWIN):
        pos = 0
        for t in range(TILES):
            R = int(R_tq[t, q])
            if R == 0:
                continue
            sched.runs.append(dict(q=q, t=t, R=R, s_lo=pos, idx=len(sched.runs)))
            pos += R
        qlen[q] = pos

    first_q, last_q = {}, {}
    for r in sched.runs:
        first_q.setdefault(r["t"], r["q"])
        last_q[r["t"]] = r["q"]
    sched.first_q, sched.last_q = first_q, last_q

    # runs of each q sorted by s_lo already
    runs_by_q = [[r for r in sched.runs if r["q"] == q] for q in range(NWIN)]

    sched.calls = []
    m_idx = 0
    idxc = 0
    for q in range(NWIN):
        rq = runs_by_q[q]
        pos = 0
        ri = 0
        while pos < qlen[q]:
            n = min(GCALL, qlen[q] - pos)
            call = dict(q=q, lo=pos, n=n, idx_off16=idxc, chunks=[])
            idxc += n // 16
            nch = (n + 127) // 128
            for k in range(nch):
                c_lo = pos + k * 128
                c_hi = min(pos + (k + 1) * 128, pos + n)
                segs = []
                # advance ri to first run overlapping c_lo
                while ri < len(rq) and rq[ri]["s_lo"] + rq[ri]["R"] <= c_lo:
                    ri += 1
                rj = ri
                while rj < len(rq) and rq[rj]["s_lo"] < c_hi:
                    r = rq[rj]
                    e_lo = max(r["s_lo"], c_lo) - c_lo
                    e_hi = min(r["s_lo"] + r["R"], c_hi) - c_lo
                    segs.append(dict(
                        run=r, e_lo=e_lo, e_hi=e_hi, m_idx=m_idx,
                        start=(max(r["s_lo"], c_lo) == r["s_lo"]),
                        stop=(min(r["s_lo"] + r["R"], c_hi) == r["s_lo"] + r["R"]),
                    ))
                    m_idx += 1
                    rj += 1
                call["chunks"].append(dict(slot=k, segs=segs))
            sched.calls.append(call)
            pos += n
    sched.MX = m_idx
    sched.IDXC = idxc
    sched.max_nm = max(sum(len(c["segs"]) for c in call["chunks"])
                       for call in sched.calls)
    sched.qlen = qlen
    return sched


def preprocess(x, edge_index, n_real):
    src = np.asarray(edge_index[0], dtype=np.int64)
    dst = np.asarray(edge_index[1], dtype=np.int64)

    deg = np.bincount(dst, minlength=NT).astype(np.float32) + 2.0
    dinv = 1.0 / np.sqrt(deg)
    coef_a = (dinv[src] * dinv[dst]).astype(np.float32)

    core = dst // OWN
    dstl_a = dst - core * OWN
    t_a = dstl_a >> 7
    dloc_a = (dstl_a & 127).astype(np.int64)
    q_a = np.minimum(src // WIN, NWIN - 1)
    idxrel_a = (src - q_a * WIN).astype(np.int64)
    assert idxrel_a.max() < 32768

    counts = np.zeros((NCORES, TILES, NWIN), dtype=np.int64)
    np.add.at(counts, (core, t_a, q_a), 1)
    R_tq = ((counts.max(axis=0) + 15) // 16 * 16).astype(np.int64)

    sched = make_schedule(R_tq)

    order = np.lexsort((idxrel_a, t_a, q_a, core))
    src_s = idxrel_a[order]
    core_s = core[order]
    t_s = t_a[order]
    q_s = q_a[order]
    dloc_s = dloc_a[order]
    coef_s = coef_a[order]

    run_pos = {(r["q"], r["t"]): r for r in sched.runs}
    # stream-global base per q
    qbase = np.cumsum([0] + sched.qlen[:-1])

    per_core = []
    for c in range(NCORES):
        sel = core_s == c
        ci, ct, cq = src_s[sel], t_s[sel], q_s[sel]
        cd, cc = dloc_s[sel], coef_s[sel]
        # flat global stream of idx / dloc / coef (padded)
        SL = int(sum(sched.qlen))
        idx_flat = np.zeros(SL, dtype=np.int16)
        dl_flat = np.zeros(SL, dtype=np.int64)
        cf_flat = np.zeros(SL, dtype=np.float32)
        key = cq * TILES + ct
        bounds = np.flatnonzero(np.r_[True, key[1:] != key[:-1], True])
        for bi in range(len(bounds) - 1):
            lo, hi = bounds[bi], bounds[bi + 1]
            r = run_pos[(int(cq[lo]), int(ct[lo]))]
            n = hi - lo
            assert n <= r["R"]
            g0 = qbase[r["q"]] + r["s_lo"]
            idx_flat[g0:g0 + n] = ci[lo:hi]
            dl_flat[g0:g0 + n] = cd[lo:hi]
            cf_flat[g0:g0 + n] = cc[lo:hi]
        # idx wrapped per call
        idx_w = np.zeros((128, sched.IDXC), dtype=np.int16)
        for call in sched.calls:
            g0 = qbase[call["q"]] + call["lo"]
            blk = idx_flat[g0:g0 + call["n"]].reshape(-1, 16).T
            o = call["idx_off16"]
            idx_w[:, o:o + call["n"] // 16] = np.tile(blk, (8, 1))
        # M tiles per segment
        M = np.zeros((sched.MX, 128, 128), dtype=np.float32)
        for call in sched.calls:
            g0 = qbase[call["q"]] + call["lo"]
            for ch in call["chunks"]:
                c_lo = g0 + ch["slot"] * 128
                for s in ch["segs"]:
                    e = np.arange(s["e_lo"], s["e_hi"])
                    gpos = c_lo + e
                    m = M[s["m_idx"]]
                    m[e, dl_flat[gpos]] = cf_flat[gpos]
        M_w = M.transpose(1, 0, 2).reshape(128, sched.MX * 128)
        own = slice(c * OWN, (c + 1) * OWN)
        selfw = (2.0 * dinv[own] * dinv[own]).astype(np.float32)
        per_core.append({
            "idx16": np.ascontiguousarray(idx_w),
            "M": np.ascontiguousarray(M_w),
            "selfw": selfw.reshape(TILES, 128).T.copy(),
        })

    return per_core, sched


# --------------------------------------------------------------------------
# bass program
# --------------------------------------------------------------------------

def build(sched):
    import os as _os0
    bf16 = _os0.environ.get("KDT", "f32") == "bf16"
    BDT = mybir.dt.bfloat16 if bf16 else mybir.dt.float32
    nc = bacc.Bacc("TRN2", target_bir_lowering=False, debug=False,
                   num_devices=NCORES, num_swdge_queues=NQUSE)

    MX, IDXC = sched.MX, sched.IDXC
    xT_in = nc.dram_tensor("xT", [128, OWN], mybir.dt.float32, kind="ExternalInput")
    idx16_in = nc.dram_tensor("idx16", [128, IDXC], mybir.dt.int16, kind="ExternalInput")
    m_in = nc.dram_tensor("M", [128, MX * 128], BDT, kind="ExternalInput")
    selfw_in = nc.dram_tensor("selfw", [128, TILES], mybir.dt.float32, kind="ExternalInput")
    w_in = [nc.dram_tensor(f"W{l}", [128, 128], mybir.dt.float32, kind="ExternalInput")
            for l in range(4)]
    b_in = [nc.dram_tensor(f"b{l}", [128, 128], mybir.dt.float32, kind="ExternalInput")
            for l in range(4)]
    ident_in = nc.dram_tensor("ident", [128, 128], mybir.dt.float32, kind="ExternalInput")
    out_dram = nc.dram_tensor("out", [OWN, 128], mybir.dt.float32, kind="ExternalOutput")

    max_call_chunks = max((c["n"] + 127) // 128 for c in sched.calls)
    # M tiles per call
    call_m0 = []
    for call in sched.calls:
        first_seg = call["chunks"][0]["segs"][0]["m_idx"]
        nm = sum(len(ch["segs"]) for ch in call["chunks"])
        call_m0.append((first_seg, nm))
    max_nm = sched.max_nm

    with tile.TileContext(nc) as tc:
        with (
            tc.tile_pool(name="persist", bufs=1) as pers,
            tc.tile_pool(name="ybuf", bufs=YBUFS) as yp,
            tc.tile_pool(name="mbuf", bufs=MBUFS) as mp,
            tc.tile_pool(name="ibuf", bufs=4) as ip,
            tc.tile_pool(name="runp", bufs=6, space="PSUM") as rp,
            tc.tile_pool(name="epip", bufs=1, space="PSUM") as ep,
            tc.tile_pool(name="etmp", bufs=4) as et,
            tc.tile_pool(name="xtile", bufs=4) as xp,
            tc.tile_pool(name="dram", bufs=1, space="DRAM") as dp,
        ):
            # ---- persistent SBUF ----
            selfw_t = pers.tile([128, TILES], mybir.dt.float32, tag="selfw")
            nc.sync.dma_start(selfw_t[:], selfw_in[:])
            ident_t = pers.tile([128, 128], mybir.dt.float32, tag="ident")
            nc.sync.dma_start(ident_t[:], ident_in[:])
            w_t, b_t = [], []
            for l in range(4):
                wt = pers.tile([128, 128], mybir.dt.float32, tag=f"w{l}")
                nc.sync.dma_start(wt[:], w_in[l][:])
                w_t.append(wt)
                bt = pers.tile([128, 128], mybir.dt.float32, tag=f"b{l}")
                nc.sync.dma_start(bt[:], b_in[l][:])
                b_t.append(bt)
            agg_t = pers.tile([128, TILES * 128], mybir.dt.float32, tag="agg")
            g_t = pers.tile([128, TILES * 128], mybir.dt.float32, tag="g")

            # zero gather slots once (short-count gathers leave stale tails;
            # M zero rows null them unless stale bits are NaN)
            y_static = []
            for _ in range(YBUFS):
                yz = yp.tile([128, max_call_chunks, 128], BDT, tag="y")
                nc.vector.memset(yz[:], 0.0)
                y_static.append(yz)

            # ---- collective buffers ----
            cc_in = [dp.tile([OWN, 128], BDT, tag=f"ccin{l}",
                             name=f"ccin{l}") for l in range(4)]
            import os as _os
            _reps = int(_os.environ.get("KREPS", "1"))
            _skip_gather = _os.environ.get("KSKIP_GATHER") == "1"
            _skip_cc = _os.environ.get("KSKIP_CC") == "1"
            _pe_light = _os.environ.get("KPE_LIGHT") == "1"
            cc_out = [dp.tile([NT, 128], BDT, tag=f"ccout{i}",
                              name=f"ccout{i}", addr_space="Shared")
                      for i in range(4 * _reps)]

            def make_xw(l, lhsT_tile, t):
                pxw = ep.tile([128, 128], mybir.dt.float32, space="PSUM", tag="pxw", bufs=1)
                nc.tensor.matmul(out=pxw[:], lhsT=lhsT_tile[:], rhs=w_t[l][:],
                                 start=True, stop=True)
                xw_sb = et.tile([128, 128], BDT, tag="xwsb")
                nc.vector.tensor_copy(xw_sb[:], pxw[:])
                nc.sync.dma_start(cc_in[l][t * 128:(t + 1) * 128, :], xw_sb[:])

            # ---- layer 0 pre-phase: xw0 = x @ W0 ----
            for t in range(TILES):
                xt = xp.tile([128, 128], mybir.dt.float32, tag="xt")
                nc.sync.dma_start(xt[:], xT_in[:, t * 128:(t + 1) * 128])
                make_xw(0, xt, t)

            gq = [0]

            def epilogue(l, t):
                agg_sl = agg_t[:, t * 128:(t + 1) * 128]
                g_sl = g_t[:, t * 128:(t + 1) * 128]
                xwown = xp.tile([128, 128], BDT, tag="xwown")
                nc.sync.dma_start(xwown[:], cc_in[l][t * 128:(t + 1) * 128, :])
                selfh = et.tile([128, 128], mybir.dt.float32, tag="selfh")
                nc.vector.tensor_scalar_mul(selfh[:], xwown[:],
                                            selfw_t[:, t:t + 1])
                h = et.tile([128, 128], mybir.dt.float32, tag="h")
                nc.vector.tensor_tensor(out=h[:], in0=agg_sl, in1=selfh[:],
                                        op=mybir.AluOpType.add)
                nc.vector.tensor_tensor(out=h[:], in0=h[:], in1=b_t[l][:],
                                        op=mybir.AluOpType.add)
                if l in (1, 2):
                    nc.vector.tensor_tensor(out=h[:], in0=h[:], in1=g_sl,
                                            op=mybir.AluOpType.add)
                if l == 3:
                    nc.sync.dma_start(out_dram[t * 128:(t + 1) * 128, :], h[:])
                    return
                nc.scalar.activation(g_sl, h[:],
                                     mybir.ActivationFunctionType.Gelu)
                pgt = ep.tile([128, 128], mybir.dt.float32, space="PSUM",
                              tag="pgt", bufs=1)
                nc.tensor.transpose(out=pgt[:], in_=g_sl, identity=ident_t[:])
                gt_sb = et.tile([128, 128], mybir.dt.float32, tag="gt")
                nc.vector.tensor_copy(gt_sb[:], pgt[:])
                make_xw(l + 1, gt_sb, t)

            def do_layer(l, rep=0):
                if not _skip_cc:
                    nc.gpsimd.collective_compute(
                        "AllGather",
                        mybir.AluOpType.bypass,
                        replica_groups=[list(range(NCORES))],
                        ins=[cc_in[l][:].opt()],
                        outs=[cc_out[rep * 4 + l][:].opt()],
                    )
                    table = cc_out[rep * 4 + l][:]
                else:
                    table = m_in[:].rearrange("p (a b) -> (p a) b", b=128)
                psum_of_run = {}
                for ci, call in enumerate(sched.calls):
                    q, n = call["q"], call["n"]
                    nch = (n + 127) // 128
                    nwin_rows = min(32768, NT - q * WIN)
                    o16 = call["idx_off16"]
                    if _skip_gather:
                        y = y_static[ci % YBUFS]
                    else:
                        y = yp.tile([128, max_call_chunks, 128], BDT,
                                    tag="y")
                        idxs = ip.tile([128, max(GCALL // 16, 16)],
                                       mybir.dt.int16, tag="idxs")
                        nc.sync.dma_start(idxs[:, :n // 16],
                                          idx16_in[:, o16:o16 + n // 16])
                        nc.gpsimd.dma_gather(
                            out_ap=y[:, :nch, :],
                            in_ap=table[q * WIN:q * WIN + nwin_rows, :],
                            idxs_ap=idxs[:, :n // 16],
                            num_idxs=n,
                            num_idxs_reg=n,
                            elem_size=128,
                            single_packet=False,
                            queue_num=gq[0] % NQUSE,
                        )
                        gq[0] += 1
                    m0, nm = call_m0[ci]
                    ms = mp.tile([128, max_nm * 128], BDT, tag="ms")
                    nc.sync.dma_start(ms[:, :nm * 128],
                                      m_in[:, m0 * 128:(m0 + nm) * 128])
                    for ch in call["chunks"]:
                        k = ch["slot"]
                        for s in ch["segs"]:
                            r = s["run"]
                            rid = r["idx"]
                            if s["start"]:
                                psum_of_run[rid] = rp.tile(
                                    [128, 128], mybir.dt.float32,
                                    space="PSUM", tag="rp", name="rpt")
                            psum = psum_of_run[rid]
                            mi = s["m_idx"] - m0
                            if not _pe_light or s["start"] or s["stop"]:
                                nc.tensor.matmul(
                                    out=psum[:],
                                    lhsT=ms[:, mi * 128:(mi + 1) * 128],
                                    rhs=y[:, k, :],
                                    start=s["start"], stop=s["stop"])
                            if s["stop"]:
                                t = r["t"]
                                agg_sl = agg_t[:, t * 128:(t + 1) * 128]
                                if q == sched.first_q[t]:
                                    nc.vector.tensor_copy(agg_sl, psum[:])
                                else:
                                    nc.vector.tensor_tensor(
                                        out=agg_sl, in0=agg_sl, in1=psum[:],
                                        op=mybir.AluOpType.add)
                                del psum_of_run[rid]
                                if q == sched.last_q[t]:
                                    epilogue(l, t)

            for rep in range(_reps):
                for l in range(4):
                    do_layer(l, rep)

    nc.compile()
    return nc


# --------------------------------------------------------------------------
# public entry point
# --------------------------------------------------------------------------

def _host_inputs(x, edge_index, Ws, bs):
    n_real = x.shape[0]
    per_core, sched = preprocess(x, edge_index, n_real)

    xpad = np.zeros((NT, F), dtype=np.float32)
    xpad[:n_real] = np.asarray(x, dtype=np.float32)

    W3p = np.zeros((128, 128), np.float32)
    W3p[:, :C_OUT] = Ws[3]
    Wl = [np.asarray(Ws[0], np.float32), np.asarray(Ws[1], np.float32),
          np.asarray(Ws[2], np.float32), W3p]
    b3p = np.zeros(128, np.float32)
    b3p[:C_OUT] = bs[3]
    bl = [np.asarray(bs[0], np.float32), np.asarray(bs[1], np.float32),
          np.asarray(bs[2], np.float32), b3p]

    ident = np.eye(128, dtype=np.float32)

    import os as _os1
    if _os1.environ.get("KDT", "f32") == "bf16":
        import ml_dtypes
        for d in per_core:
            d["M"] = d["M"].astype(ml_dtypes.bfloat16)
    in_maps = []
    for c in range(NCORES):
        d = per_core[c]
        m = {
            "xT": xpad[c * OWN:(c + 1) * OWN].T.copy(),
            "idx16": d["idx16"],
            "M": d["M"],
            "selfw": d["selfw"],
            "ident": ident,
        }
        for l in range(4):
            m[f"W{l}"] = Wl[l]
            m[f"b{l}"] = np.tile(bl[l], (128, 1))
        in_maps.append(m)
    return in_maps, sched


def kernel(x, edge_index, W0, b0, W1, b1, W2, b2, W3, b3):
    x = np.asarray(x)
    in_maps, sched = _host_inputs(
        x, np.asarray(edge_index), [W0, W1, W2, W3], [b0, b1, b2, b3])
    nc = build(sched)
    res = run_bass_kernel_spmd(nc, in_maps, list(range(NCORES)))
    outs = [res.results[c]["out"] for c in range(NCORES)]
    full = np.concatenate(outs, axis=0)[:x.shape[0], :C_OUT]
    return full.astype(np.float32)



# revision 4
# speedup vs baseline: 1.1200x; 1.1200x over previous
"""GCN node classification on 8 Trainium2 NeuronCores (Bass/Tile).

Strategy (dst-sharded graph parallel), v4:
  - Nodes padded to 100352 = 8 * 12544; core c owns dst nodes
    [c*12544, (c+1)*12544)  (98 tiles of 128).
  - Per layer: each core computes xw = g_own @ W on PE; an AllGather makes
    the full [100352, F] bf16 feature table resident on every core's HBM.
  - Self-loop terms are folded into the edge list as explicit self-edges
    (coef = 2*dinv^2), so aggregation is one uniform gather+matmul pass.
  - Edges are bucketed by (dst-tile, 25088-row src window = "run"), sorted
    by src; run lengths are the max over the 8 cores (SPMD-uniform
    schedule), rounded to 16; shorter cores pad with idx 0 and zero rows
    in M. Runs are packed into one index stream per window; dma_gather
    calls of up to 2048 idxs pull source rows (int16 idx, relative to the
    window; the full idx stream is SBUF-resident). Host-precomputed
    selection matrices M[e,d] = coef[e] * (d == dst_local[e]) (bf16) are
    DMA-streamed from HBM, and PE accumulates psum += Y^T @ M (layers
    0-2: output [feat, dst], transposed epilogue) or psum += M^T @ Y
    (layer 3: [dst, feat] for the node-major output).
  - Epilogue per tile (layers 0-2, feat-major): +residual (DVE),
    gelu(agg + bias) in one ACT op with per-partition bias, then the next
    layer's matmul directly from the feat-major activation (no transpose
    needed), DMA into the next collective's input buffer.
"""
import sys

sys.path.insert(0, "/opt/trn_rl_repo")

import numpy as np

import concourse.bass as bass  # noqa: E402
import concourse.tile as tile  # noqa: E402
from concourse import bacc, mybir  # noqa: E402
from concourse.bass_utils import run_bass_kernel_spmd  # noqa: E402

NCORES = 8
F = 128          # feature width (all layers padded to 128)
TILES = 98       # dst tiles per core
OWN = TILES * 128            # 12544 nodes per core
NT = NCORES * OWN            # 100352 padded nodes
NWIN = 4
WIN = 25088                  # src window (int16-addressable, < 32768)
GCALL = 2048                 # idxs per dma_gather call
C_OUT = 40
YBUFS = 5
MBUFS = 3
NQUSE = 4


# --------------------------------------------------------------------------
# host-side schedule
# --------------------------------------------------------------------------

class Sched:
    """Shared (core-independent) schedule.

    runs:  list of dicts {q, t, R, s_lo (stream pos within q), first/last}
    calls: list of dicts {q, lo, n, chunks: [ {slot, segs: [
               {run_idx, e_lo, e_hi, m_idx, start, stop} ] } ]}
    MX:    total number of M tiles
    IDXC:  int16 idx columns (sum over calls of n/16)
    """


def make_schedule(R_tq):
    sched = Sched()
    sched.runs = []
    qlen = [0] * NWIN
    for q in range(NWIN):
        pos = 0
        for t in range(TILES):
            R = int(R_tq[t, q])
            if R == 0:
                continue
            sched.runs.append(dict(q=q, t=t, R=R, s_lo=pos, idx=len(sched.runs)))
            pos += R
        qlen[q] = pos

    first_q, last_q = {}, {}
    for r in sched.runs:
        first_q.setdefault(r["t"], r["q"])
        last_q[r["t"]] = r["q"]
    sched.first_q, sched.last_q = first_q, last_q

    # runs of each q sorted by s_lo already
    runs_by_q = [[r for r in sched.runs if r["q"] == q] for q in range(NWIN)]

    sched.calls = []
    m_idx = 0
    idxc = 0
    for q in range(NWIN):
        rq = runs_by_q[q]
        pos = 0
        ri = 0
        while pos < qlen[q]:
            n = min(GCALL, qlen[q] - pos)
            call = dict(q=q, lo=pos, n=n, idx_off16=idxc, chunks=[])
            idxc += n // 16
            nch = (n + 127) // 128
            for k in range(nch):
                c_lo = pos + k * 128
                c_hi = min(pos + (k + 1) * 128, pos + n)
                segs = []
                # advance ri to first run overlapping c_lo
                while ri < len(rq) and rq[ri]["s_lo"] + rq[ri]["R"] <= c_lo:
                    ri += 1
                rj = ri
                while rj < len(rq) and rq[rj]["s_lo"] < c_hi:
                    r = rq[rj]
                    e_lo = max(r["s_lo"], c_lo) - c_lo
                    e_hi = min(r["s_lo"] + r["R"], c_hi) - c_lo
                    segs.append(dict(
                        run=r, e_lo=e_lo, e_hi=e_hi, m_idx=m_idx,
                        start=(max(r["s_lo"], c_lo) == r["s_lo"]),
                        stop=(min(r["s_lo"] + r["R"], c_hi) == r["s_lo"] + r["R"]),
                    ))
                    m_idx += 1
                    rj += 1
                call["chunks"].append(dict(slot=k, segs=segs))
            sched.calls.append(call)
            pos += n
    sched.MX = m_idx
    sched.IDXC = idxc
    sched.max_nm = max(sum(len(c["segs"]) for c in call["chunks"])
                       for call in sched.calls)
    sched.qlen = qlen
    return sched


def preprocess(x, edge_index, n_real):
    src_r = np.asarray(edge_index[0], dtype=np.int64)
    dst_r = np.asarray(edge_index[1], dtype=np.int64)

    deg = np.bincount(dst_r, minlength=NT).astype(np.float32) + 2.0
    dinv = 1.0 / np.sqrt(deg)

    # fold the self-loop term into the edge list: one self-edge per node
    # with coef = 2*dinv^2 (PyG improved=True self-loop weight).
    allv = np.arange(NT, dtype=np.int64)
    src = np.concatenate([src_r, allv])
    dst = np.concatenate([dst_r, allv])
    coef_a = np.concatenate([
        (dinv[src_r] * dinv[dst_r]).astype(np.float32),
        (2.0 * dinv * dinv).astype(np.float32),
    ])

    core = dst // OWN
    dstl_a = dst - core * OWN
    t_a = dstl_a >> 7
    dloc_a = (dstl_a & 127).astype(np.int64)
    q_a = np.minimum(src // WIN, NWIN - 1)
    idxrel_a = (src - q_a * WIN).astype(np.int64)
    assert idxrel_a.max() < 32768

    counts = np.zeros((NCORES, TILES, NWIN), dtype=np.int64)
    np.add.at(counts, (core, t_a, q_a), 1)
    R_tq = ((counts.max(axis=0) + 15) // 16 * 16).astype(np.int64)

    sched = make_schedule(R_tq)

    order = np.lexsort((idxrel_a, t_a, q_a, core))
    src_s = idxrel_a[order]
    core_s = core[order]
    t_s = t_a[order]
    q_s = q_a[order]
    dloc_s = dloc_a[order]
    coef_s = coef_a[order]

    run_pos = {(r["q"], r["t"]): r for r in sched.runs}
    # stream-global base per q
    qbase = np.cumsum([0] + sched.qlen[:-1])

    per_core = []
    for c in range(NCORES):
        sel = core_s == c
        ci, ct, cq = src_s[sel], t_s[sel], q_s[sel]
        cd, cc = dloc_s[sel], coef_s[sel]
        # flat global stream of idx / dloc / coef (padded)
        SL = int(sum(sched.qlen))
        idx_flat = np.zeros(SL, dtype=np.int16)
        dl_flat = np.zeros(SL, dtype=np.int64)
        cf_flat = np.zeros(SL, dtype=np.float32)
        key = cq * TILES + ct
        bounds = np.flatnonzero(np.r_[True, key[1:] != key[:-1], True])
        for bi in range(len(bounds) - 1):
            lo, hi = bounds[bi], bounds[bi + 1]
            r = run_pos[(int(cq[lo]), int(ct[lo]))]
            n = hi - lo
            assert n <= r["R"]
            g0 = qbase[r["q"]] + r["s_lo"]
            idx_flat[g0:g0 + n] = ci[lo:hi]
            dl_flat[g0:g0 + n] = cd[lo:hi]
            cf_flat[g0:g0 + n] = cc[lo:hi]
        # idx wrapped per call
        idx_w = np.zeros((128, sched.IDXC), dtype=np.int16)
        for call in sched.calls:
            g0 = qbase[call["q"]] + call["lo"]
            blk = idx_flat[g0:g0 + call["n"]].reshape(-1, 16).T
            o = call["idx_off16"]
            idx_w[:, o:o + call["n"] // 16] = np.tile(blk, (8, 1))
        # M tiles per segment
        M = np.zeros((sched.MX, 128, 128), dtype=np.float32)
        for call in sched.calls:
            g0 = qbase[call["q"]] + call["lo"]
            for ch in call["chunks"]:
                c_lo = g0 + ch["slot"] * 128
                for s in ch["segs"]:
                    e = np.arange(s["e_lo"], s["e_hi"])
                    gpos = c_lo + e
                    m = M[s["m_idx"]]
                    m[e, dl_flat[gpos]] = cf_flat[gpos]
        M_w = M.transpose(1, 0, 2).reshape(128, sched.MX * 128)
        per_core.append({
            "idx16": np.ascontiguousarray(idx_w),
            "M": np.ascontiguousarray(M_w),
        })

    return per_core, sched


# --------------------------------------------------------------------------
# bass program
# --------------------------------------------------------------------------

def build(sched):
    BDT = mybir.dt.bfloat16
    nc = bacc.Bacc("TRN2", target_bir_lowering=False, debug=False,
                   num_devices=NCORES, num_swdge_queues=NQUSE)

    MX, IDXC = sched.MX, sched.IDXC
    xT_in = nc.dram_tensor("xT", [128, OWN], BDT, kind="ExternalInput")
    idx16_in = nc.dram_tensor("idx16", [128, IDXC], mybir.dt.int16, kind="ExternalInput")
    m_in = nc.dram_tensor("M", [128, MX * 128], BDT, kind="ExternalInput")
    w_in = [nc.dram_tensor(f"W{l}", [128, 128], BDT, kind="ExternalInput")
            for l in range(4)]
    bcol_in = [nc.dram_tensor(f"bc{l}", [128, 1], mybir.dt.float32,
                              kind="ExternalInput") for l in range(3)]
    b3_in = nc.dram_tensor("b3t", [128, 128], mybir.dt.float32, kind="ExternalInput")
    out_dram = nc.dram_tensor("out", [OWN, 128], mybir.dt.float32, kind="ExternalOutput")

    max_call_chunks = max((c["n"] + 127) // 128 for c in sched.calls)
    # M tiles per call
    call_m0 = []
    for call in sched.calls:
        first_seg = call["chunks"][0]["segs"][0]["m_idx"]
        nm = sum(len(ch["segs"]) for ch in call["chunks"])
        call_m0.append((first_seg, nm))
    max_nm = sched.max_nm

    with tile.TileContext(nc) as tc:
        with (
            tc.tile_pool(name="persist", bufs=1) as pers,
            tc.tile_pool(name="ybuf", bufs=YBUFS) as yp,
            tc.tile_pool(name="mbuf", bufs=MBUFS) as mp,
            tc.tile_pool(name="runp", bufs=6, space="PSUM") as rp,
            tc.tile_pool(name="epip", bufs=1, space="PSUM") as ep,
            tc.tile_pool(name="etmp", bufs=4) as et,
            tc.tile_pool(name="dram", bufs=1, space="DRAM") as dp,
        ):
            # ---- persistent SBUF ----
            idx_t = pers.tile([128, IDXC], mybir.dt.int16, tag="idx")
            nc.sync.dma_start(idx_t[:], idx16_in[:])
            xT_t = pers.tile([128, OWN], BDT, tag="xT")
            nc.sync.dma_start(xT_t[:], xT_in[:])
            w_t, bc_t = [], []
            for l in range(4):
                wt = pers.tile([128, 128], BDT, tag=f"w{l}")
                nc.sync.dma_start(wt[:], w_in[l][:])
                w_t.append(wt)
            for l in range(3):
                bt = pers.tile([128, 1], mybir.dt.float32, tag=f"bc{l}")
                nc.sync.dma_start(bt[:], bcol_in[l][:])
                bc_t.append(bt)
            b3_t = pers.tile([128, 128], mybir.dt.float32, tag="b3")
            nc.sync.dma_start(b3_t[:], b3_in[:])
            agg_t = pers.tile([128, TILES * 128], mybir.dt.float32, tag="agg")
            g_t = pers.tile([128, TILES * 128], BDT, tag="g")

            # zero gather slots once (short-count gathers leave stale tails;
            # M zero rows null them unless stale bits are NaN)
            for _ in range(YBUFS):
                yz = yp.tile([128, max_call_chunks, 128], BDT, tag="y")
                nc.vector.memset(yz[:], 0.0)

            # ---- collective buffers ----
            cc_in = [dp.tile([OWN, 128], BDT, tag=f"ccin{l}",
                             name=f"ccin{l}") for l in range(4)]
            cc_out = [dp.tile([NT, 128], BDT, tag=f"ccout{i}",
                              name=f"ccout{i}", addr_space="Shared")
                      for i in range(4)]

            def make_xw(l, t):
                lhsT = (xT_t if l == 0 else g_t)[:, t * 128:(t + 1) * 128]
                pxw = ep.tile([128, 128], mybir.dt.float32, space="PSUM", tag="pxw", bufs=1)
                nc.tensor.matmul(out=pxw[:], lhsT=lhsT, rhs=w_t[l][:],
                                 start=True, stop=True)
                xw_sb = et.tile([128, 128], BDT, tag="xwsb")
                nc.vector.tensor_copy(xw_sb[:], pxw[:])
                nc.sync.dma_start(cc_in[l][t * 128:(t + 1) * 128, :], xw_sb[:])

            # ---- layer 0 pre-phase: xw0 = x @ W0 ----
            for t in range(TILES):
                make_xw(0, t)

            gq = [0]

            def epilogue(l, t):
                agg_sl = agg_t[:, t * 128:(t + 1) * 128]
                g_sl = g_t[:, t * 128:(t + 1) * 128]
                if l == 3:
                    h = et.tile([128, 128], mybir.dt.float32, tag="h")
                    nc.vector.tensor_tensor(out=h[:], in0=agg_sl, in1=b3_t[:],
                                            op=mybir.AluOpType.add)
                    nc.sync.dma_start(out_dram[t * 128:(t + 1) * 128, :], h[:])
                    return
                if l in (1, 2):
                    nc.vector.tensor_tensor(out=agg_sl, in0=agg_sl, in1=g_sl,
                                            op=mybir.AluOpType.add)
                nc.scalar.activation(g_sl, agg_sl,
                                     mybir.ActivationFunctionType.Gelu,
                                     bias=bc_t[l][:])
                make_xw(l + 1, t)

            def do_layer(l):
                nc.gpsimd.collective_compute(
                    "AllGather",
                    mybir.AluOpType.bypass,
                    replica_groups=[list(range(NCORES))],
                    ins=[cc_in[l][:].opt()],
                    outs=[cc_out[l][:].opt()],
                )
                table = cc_out[l][:]
                psum_of_run = {}
                for ci, call in enumerate(sched.calls):
                    q, n = call["q"], call["n"]
                    nch = (n + 127) // 128
                    nwin_rows = min(32768, NT - q * WIN)
                    o16 = call["idx_off16"]
                    y = yp.tile([128, max_call_chunks, 128], BDT, tag="y")
                    nc.gpsimd.dma_gather(
                        out_ap=y[:, :nch, :],
                        in_ap=table[q * WIN:q * WIN + nwin_rows, :],
                        idxs_ap=idx_t[:, o16:o16 + n // 16],
                        num_idxs=n,
                        num_idxs_reg=n,
                        elem_size=128,
                        single_packet=False,
                        queue_num=gq[0] % NQUSE,
                    )
                    gq[0] += 1
                    m0, nm = call_m0[ci]
                    ms = mp.tile([128, max_nm * 128], BDT, tag="ms")
                    nc.scalar.dma_start(ms[:, :nm * 128],
                                        m_in[:, m0 * 128:(m0 + nm) * 128])
                    for ch in call["chunks"]:
                        k = ch["slot"]
                        for s in ch["segs"]:
                            r = s["run"]
                            rid = r["idx"]
                            if s["start"]:
                                psum_of_run[rid] = rp.tile(
                                    [128, 128], mybir.dt.float32,
                                    space="PSUM", tag="rp", name="rpt")
                            psum = psum_of_run[rid]
                            mi = s["m_idx"] - m0
                            if l == 3:
                                nc.tensor.matmul(
                                    out=psum[:],
                                    lhsT=ms[:, mi * 128:(mi + 1) * 128],
                                    rhs=y[:, k, :],
                                    start=s["start"], stop=s["stop"])
                            else:
                                nc.tensor.matmul(
                                    out=psum[:],
                                    lhsT=y[:, k, :],
                                    rhs=ms[:, mi * 128:(mi + 1) * 128],
                                    start=s["start"], stop=s["stop"])
                            if s["stop"]:
                                t = r["t"]
                                agg_sl = agg_t[:, t * 128:(t + 1) * 128]
                                if q == sched.first_q[t]:
                                    nc.scalar.copy(agg_sl, psum[:])
                                else:
                                    nc.vector.tensor_tensor(
                                        out=agg_sl, in0=agg_sl, in1=psum[:],
                                        op=mybir.AluOpType.add)
                                del psum_of_run[rid]
                                if q == sched.last_q[t]:
                                    epilogue(l, t)

            for l in range(4):
                do_layer(l)

    nc.compile()
    return nc


# --------------------------------------------------------------------------
# public entry point
# --------------------------------------------------------------------------

def _host_inputs(x, edge_index, Ws, bs):
    import ml_dtypes
    n_real = x.shape[0]
    per_core, sched = preprocess(x, edge_index, n_real)

    xpad = np.zeros((NT, F), dtype=np.float32)
    xpad[:n_real] = np.asarray(x, dtype=np.float32)

    W3p = np.zeros((128, 128), np.float32)
    W3p[:, :C_OUT] = Ws[3]
    Wl = [np.asarray(Ws[0], np.float32), np.asarray(Ws[1], np.float32),
          np.asarray(Ws[2], np.float32), W3p]
    b3p = np.zeros(128, np.float32)
    b3p[:C_OUT] = bs[3]

    in_maps = []
    for c in range(NCORES):
        d = per_core[c]
        m = {
            "xT": xpad[c * OWN:(c + 1) * OWN].T.astype(ml_dtypes.bfloat16),
            "idx16": d["idx16"],
            "M": d["M"].astype(ml_dtypes.bfloat16),
            "b3t": np.tile(b3p, (128, 1)),
        }
        for l in range(4):
            m[f"W{l}"] = Wl[l].astype(ml_dtypes.bfloat16)
        for l in range(3):
            m[f"bc{l}"] = np.asarray(bs[l], np.float32).reshape(128, 1)
        in_maps.append(m)
    return in_maps, sched


def kernel(x, edge_index, W0, b0, W1, b1, W2, b2, W3, b3):
    x = np.asarray(x)
    in_maps, sched = _host_inputs(
        x, np.asarray(edge_index), [W0, W1, W2, W3], [b0, b1, b2, b3])
    nc = build(sched)
    res = run_bass_kernel_spmd(nc, in_maps, list(range(NCORES)))
    outs = [res.results[c]["out"] for c in range(NCORES)]
    full = np.concatenate(outs, axis=0)[:x.shape[0], :C_OUT]
    return full.astype(np.float32)
